# revision 15
# baseline (speedup 1.0000x reference)
"""DWA LanguageModel layer on 8 trn2 NeuronCores (v3).

Strategy:
  - Tokens (B=1024) data-parallel across 8 cores (128 each).
  - Keys N-sharded: each core computes keys for its 128 pool rows over
    the full D=16384 contraction, normalizes along the free dim,
    transposes, and AllGathers 32KB of bf16 normalized keysT (the
    collective is latency-bound, so ship the smallest payload).
  - Dynamic path fp8 (e4m3, scaled): never materializes UV;
    h_delta[b] = sum_nr s[b,nr] U[nr,:] with s = alpha * (z @ V^T).
    Pool bias folded into the same contraction as 8 extra chunks
    (s' = [s, 16*alpha], up' = [64*U_perm, 64*bias]); the 1024x scale
    folds into the alpha-normalization reciprocal.
  - Top-16 threshold per token via vector.max + match_replace + max.
  - All large DMAs are [128, X] contiguous transfers (host pre-packs
    the exact SBUF layout); s-mults split across DVE and GpSimd; the
    transpose+h1 loop is software-pipelined behind them.
"""
import sys

sys.path.insert(0, "/opt/trn_rl_repo")
import numpy as np
import ml_dtypes

import concourse.bass as bass
import concourse.mybir as mybir
import concourse.tile as tile
from concourse import bacc
from concourse.bass_utils import run_bass_kernel_spmd
from concourse.masks import make_identity

F32 = mybir.dt.float32
BF16 = mybir.dt.bfloat16
FP8 = mybir.dt.float8e4
AF = mybir.ActivationFunctionType
ALU = mybir.AluOpType

NCORES = 8
B = 1024            # tokens
BL = B // NCORES    # tokens per core = 128
D_MODEL = 512
N = 1024            # pool rows
NL = N // NCORES    # pool rows per core = 128
D = 16384           # pool cols
S = 2
DK = 64
SDK = S * DK        # 128
R = 8
NR = N * R          # 8192
K_MAX = 16
LAMBDA_SHARP = 5.0
LN_EPS = 1e-5
U_END = D_MODEL * R          # 4096
V_END = U_END + R * D_MODEL  # 8192
B_END = V_END + D_MODEL      # 8704

SC_V = 16.0          # scale on V^T (and on alpha chunk of s')
SC_U = 64.0          # scale on U_perm / bias
SC_H = SC_V * SC_U   # total scale on h1 psum = 1024

LAST_EXEC_NS = None
TRACE = False
TMPDIR = None
NO_CC = False
LEVEL = 9  # bisect: 0=io 3=scores 4=alpha 9=full


def _build(tau_f, w0_f, w1_f, gamma_f):
    nc = bacc.Bacc("TRN2", target_bir_lowering=False, debug=False,
                   num_devices=NCORES)

    # ---- I/O (all pre-packed to exact SBUF layout [128, X]) ----
    pk_d = nc.dram_tensor("pk", [128, 128 * NL], FP8, kind="ExternalInput")
    wk_d = nc.dram_tensor("wk", [128, 128 * SDK], FP8, kind="ExternalInput")
    wq_d = nc.dram_tensor("wq", [128, 4 * SDK], BF16, kind="ExternalInput")
    zt_d = nc.dram_tensor("zt", [128, 4 * BL], BF16, kind="ExternalInput")
    zb_d = nc.dram_tensor("zb", [BL, D_MODEL], F32, kind="ExternalInput")
    ls_d = nc.dram_tensor("ls", [BL, D_MODEL], F32, kind="ExternalInput")
    lb_d = nc.dram_tensor("lb", [BL, D_MODEL], F32, kind="ExternalInput")
    wbt_d = nc.dram_tensor("wbt", [128, 4 * D_MODEL], BF16,
                           kind="ExternalInput")
    vt_d = nc.dram_tensor("vt", [128, 4 * NR], FP8, kind="ExternalInput")
    up_d = nc.dram_tensor("up", [128, 72 * D_MODEL], FP8,
                          kind="ExternalInput")
    out_d = nc.dram_tensor("out", [BL, D_MODEL], F32, kind="ExternalOutput")

    with tile.TileContext(nc) as tc:
        with (
            tc.tile_pool(name="sb", bufs=1) as sb,
            tc.tile_pool(name="sbr", bufs=8) as sbr,     # rotating sT tiles
            tc.tile_pool(name="dram", bufs=1, space="DRAM") as dram,
        ):
            _emit(nc, tc, sb, sbr, dram, tau_f, w0_f, w1_f, gamma_f,
                  pk_d, wk_d, wq_d, zt_d, zb_d, ls_d, lb_d, wbt_d,
                  vt_d, up_d, out_d)

    nc.compile()
    return nc


def _emit(nc, tc, sb, sbr, dram, tau_f, w0_f, w1_f, gamma_f,
          pk_d, wk_d, wq_d, zt_d, zb_d, ls_d, lb_d, wbt_d,
          vt_d, up_d, out_d):
    # ---------- DMA loads, priority order (chunked for pipelining) ----------
    pk_sb = sb.tile([128, 128 * NL], FP8, tag="pk")
    wk_sb = sb.tile([128, 128 * SDK], FP8, tag="wk")
    for i in range(4):
        q4 = 32 * 128
        nc.sync.dma_start(pk_sb[:, i * q4:(i + 1) * q4],
                          pk_d[:, i * q4:(i + 1) * q4])
        nc.sync.dma_start(wk_sb[:, i * q4:(i + 1) * q4],
                          wk_d[:, i * q4:(i + 1) * q4])
    zt_sb = sb.tile([128, 4 * BL], BF16, tag="zt")
    nc.sync.dma_start(zt_sb[:], zt_d[:])
    wq_sb = sb.tile([128, 4 * SDK], BF16, tag="wq")
    nc.sync.dma_start(wq_sb[:], wq_d[:])
    zb_sb = sb.tile([BL, D_MODEL], F32, tag="zb")
    nc.sync.dma_start(zb_sb[:], zb_d[:])
    wbt_sb = sb.tile([128, 4 * D_MODEL], BF16, tag="wbt")
    nc.sync.dma_start(wbt_sb[:], wbt_d[:])
    vt_sb = sb.tile([128, 4 * NR], FP8, tag="vt")
    nc.sync.dma_start(vt_sb[:], vt_d[:])
    up_sb = sb.tile([128, 72 * D_MODEL], FP8, tag="up")
    nc.sync.dma_start(up_sb[:], up_d[:])
    ls_sb = sb.tile([BL, D_MODEL], F32, tag="ls")
    nc.sync.dma_start(ls_sb[:], ls_d[:])
    lb_sb = sb.tile([BL, D_MODEL], F32, tag="lb")
    nc.sync.dma_start(lb_sb[:], lb_d[:])

    if LEVEL <= 0:
        o0 = sb.tile([BL, D_MODEL], F32, tag="o0")
        nc.vector.tensor_scalar_mul(o0[:], zb_sb[:], 2.0)
        nc.sync.dma_start(out_d[:], o0[:])
        return

    identb = sb.tile([128, 128], BF16, tag="identb")
    make_identity(nc, identb[:])

    t_sb = sb.tile([BL, NR], BF16, tag="t")
    q_n = sb.tile([BL, SDK], BF16, tag="q_n")
    qnT = sb.tile([SDK, BL], BF16, tag="qnT")
    h2_sb = sb.tile([BL, D_MODEL], F32, tag="h2")
    knTl = sb.tile([SDK, NL], BF16, tag="knTl")
    knT = sb.tile([SDK, N], BF16, tag="knT")

    with tc.tile_pool(name="ps1", bufs=1, space="PSUM") as ps1:
        # ---------- keys for local 128 pool rows, full D contraction ----
        keys_ps = ps1.tile([NL, SDK], F32, tag="keys")
        for k in range(128):
            nc.tensor.matmul(keys_ps[:],
                             pk_sb[:, k * NL:(k + 1) * NL],
                             wk_sb[:, k * SDK:(k + 1) * SDK],
                             start=(k == 0), stop=(k == 127))
        # normalize along free dim per aspect
        ksq = sb.tile([NL, S], F32, tag="ksq")
        ksc = sb.tile([NL, SDK], F32, tag="sqscr")
        for s in range(S):
            nc.scalar.activation(ksc[:, s * DK:(s + 1) * DK],
                                 keys_ps[:, s * DK:(s + 1) * DK],
                                 AF.Square,
                                 accum_out=ksq[:, s:s + 1])
        knorm = sb.tile([NL, S], F32, tag="knorm")
        nc.scalar.activation(knorm[:], ksq[:], AF.Sqrt)
        nc.vector.tensor_scalar_add(knorm[:], knorm[:], 1e-8)
        krec = sb.tile([NL, S], F32, tag="krec")
        nc.vector.reciprocal(krec[:], knorm[:])
        kn_w = sb.tile([NL, SDK], BF16, tag="kn_w")
        for s in range(S):
            nc.vector.tensor_scalar(
                kn_w[:, s * DK:(s + 1) * DK],
                keys_ps[:, s * DK:(s + 1) * DK],
                krec[:, s:s + 1], None, op0=ALU.mult)
        kn_tp = ps1.tile([SDK, NL], BF16, tag="kn_tp")
        nc.tensor.transpose(kn_tp[:], kn_w[:], identb[:])
        nc.scalar.activation(knTl[:], kn_tp[:], AF.Copy)

        # ---------- AllGather normalized keysT (bf16, 32KB in) ----------
        cc_in = dram.tile([SDK, NL], BF16)
        cc_out = dram.tile([N, NL], BF16)
        nc.scalar.dma_start(cc_in[:], knTl[:])
        if NO_CC:
            for c in range(NCORES):
                nc.sync.dma_start(cc_out[c * SDK:(c + 1) * SDK, :], cc_in[:])
        else:
            nc.gpsimd.collective_compute(
                "AllGather", ALU.bypass,
                replica_groups=[list(range(NCORES))],
                ins=[cc_in[:].opt()], outs=[cc_out[:].opt()],
            )
        nc.scalar.dma_start(
            knT[:].rearrange("p (c n) -> p c n", c=NCORES),
            cc_out[:].rearrange("(c p) n -> p c n", p=SDK))

        # ---------- queries [b, sdk] + normalize (bf16 path) ----------
        q_ps = ps1.tile([BL, SDK], F32, tag="q")
        for c in range(4):
            nc.tensor.matmul(q_ps[:],
                             zt_sb[:, c * BL:(c + 1) * BL],
                             wq_sb[:, c * SDK:(c + 1) * SDK],
                             start=(c == 0), stop=(c == 3))
        qsq = sb.tile([BL, S], F32, tag="qsq")
        qsc = ksc  # shared scratch, phases are sequential
        for s in range(S):
            nc.scalar.activation(qsc[:, s * DK:(s + 1) * DK],
                                 q_ps[:, s * DK:(s + 1) * DK],
                                 AF.Square,
                                 accum_out=qsq[:, s:s + 1])
        qnorm = sb.tile([BL, S], F32, tag="qnorm")
        nc.scalar.activation(qnorm[:], qsq[:], AF.Sqrt)
        nc.vector.tensor_scalar_add(qnorm[:], qnorm[:], 1e-8)
        qrec = sb.tile([BL, S], F32, tag="qrec")
        nc.vector.reciprocal(qrec[:], qnorm[:])
        # fold softmax(aspect_logits) weights into q_n
        for s, w_s in ((0, w0_f), (1, w1_f)):
            nc.vector.tensor_scalar(
                q_n[:, s * DK:(s + 1) * DK],
                q_ps[:, s * DK:(s + 1) * DK],
                qrec[:, s:s + 1], float(w_s),
                op0=ALU.mult, op1=ALU.mult)

        # ---------- t = z @ V'^T  [b, nr] (independent of CC) ----------
        with tc.tile_pool(name="psT", bufs=4, space="PSUM") as psT:
            for j in range(16):
                t_ps = psT.tile([BL, 512], F32, tag="t")
                for a in range(4):
                    nc.tensor.matmul(
                        t_ps[:],
                        zt_sb[:, a * BL:(a + 1) * BL],
                        vt_sb[:, a * NR + j * 512:a * NR + (j + 1) * 512],
                        start=(a == 0), stop=(a == 3))
                nc.scalar.activation(t_sb[:, j * 512:(j + 1) * 512],
                                     t_ps[:], AF.Copy)

        # ---------- h2 = z @ W_base^T (independent of CC) ----------
        h2_ps = ps1.tile([BL, D_MODEL], F32, tag="h2p")
        for c in range(4):
            nc.tensor.matmul(h2_ps[:],
                             zt_sb[:, c * BL:(c + 1) * BL],
                             wbt_sb[:, c * D_MODEL:(c + 1) * D_MODEL],
                             start=(c == 0), stop=(c == 3))
        nc.scalar.activation(h2_sb[:], h2_ps[:], AF.Copy)

    # ---------- scores (after CC) ----------
    scores = sb.tile([BL, N], F32, tag="scores")
    with tc.tile_pool(name="psS", bufs=2, space="PSUM") as psS:
        q_tp = psS.tile([SDK, BL], BF16, tag="qtp")
        nc.tensor.transpose(q_tp[:], q_n[:], identb[:])
        nc.scalar.activation(qnT[:], q_tp[:], AF.Copy)
        for h in range(2):
            sc_ps = psS.tile([BL, 512], F32, tag="sc")
            nc.tensor.matmul(sc_ps[:], qnT[:],
                             knT[:, h * 512:(h + 1) * 512],
                             start=True, stop=True)
            nc.scalar.activation(scores[:, h * 512:(h + 1) * 512],
                                 sc_ps[:], AF.Copy)

    if LEVEL <= 3:
        nc.sync.dma_start(out_d[:], scores[:, :D_MODEL])
        return

    # ---------- top-16 threshold + alpha (unnormalized) ----------
    m8a = sb.tile([BL, 8], F32, tag="m8a")
    nc.vector.max(out=m8a[:], in_=scores[:])
    s_mr = sb.tile([BL, N], F32, tag="s_mr")
    nc.vector.match_replace(out=s_mr[:], in_to_replace=m8a[:],
                            in_values=scores[:], imm_value=-1e30)
    m8b = sb.tile([BL, 8], F32, tag="m8b")
    nc.vector.max(out=m8b[:], in_=s_mr[:])
    sig_b = sb.tile([BL, 1], F32, tag="sig_b")
    nc.vector.memset(sig_b[:], float(-LAMBDA_SHARP * tau_f))
    sig = sb.tile([BL, N], F32, tag="sig")
    nc.scalar.activation(sig[:], scores[:], AF.Sigmoid,
                         scale=LAMBDA_SHARP, bias=sig_b[:])
    ex = sb.tile([BL, N], F32, tag="ex")
    nc.scalar.activation(ex[:], scores[:], AF.Exp)
    ge = s_mr  # s_mr dead after m8b
    nc.vector.tensor_mul(ge[:], sig[:], ex[:])
    alpha = ex  # ex dead after the ge product
    den = sb.tile([BL, 1], F32, tag="den")
    nc.vector.scalar_tensor_tensor(
        out=alpha[:], in0=scores[:], scalar=m8b[:, 7:8], in1=ge[:],
        op0=ALU.is_ge, op1=ALU.mult, accum_out=den[:])
    # rden = 1 / (SC_H * (den + 1e-8)) folds the fp8 scaling
    den2 = sb.tile([BL, 1], F32, tag="den2")
    nc.vector.tensor_scalar(den2[:], den[:], float(SC_H),
                            float(SC_H * 1e-8), op0=ALU.mult, op1=ALU.add)
    rden = sb.tile([BL, 1], F32, tag="rden")
    nc.vector.reciprocal(rden[:], den2[:])

    if LEVEL <= 4:
        nc.sync.dma_start(out_d[:], alpha[:, :D_MODEL])
        return

    # ---------- s' = [alpha*t, SC_V*alpha] bf16; pipelined h1 ----------
    s_sb = sb.tile([BL, NR + N], BF16, tag="s_sb")
    NG = 72

    def emit_smult(j):
        if j < 16:
            eng = nc.gpsimd if j % 3 == 2 else nc.vector
            eng.tensor_tensor(
                out=s_sb[:, j * 512:(j + 1) * 512]
                    .rearrange("p (n r) -> p n r", r=R),
                in0=t_sb[:, j * 512:(j + 1) * 512]
                    .rearrange("p (n r) -> p n r", r=R),
                in1=alpha[:, j * 64:(j + 1) * 64]
                    .unsqueeze(2).broadcast_to([BL, 64, R]),
                op=ALU.mult)
        else:  # alpha chunk of s'
            h = j - 16
            eng = nc.vector
            eng.tensor_scalar_mul(
                s_sb[:, NR + h * 512:NR + (h + 1) * 512],
                alpha[:, h * 512:(h + 1) * 512], float(SC_V))

    with tc.tile_pool(name="psH", bufs=1, space="PSUM") as psH, \
         tc.tile_pool(name="psR", bufs=4, space="PSUM") as psR:
        h1_ps = psH.tile([BL, D_MODEL], F32, tag="h1")
        sTs = []

        def emit_transpose(g):
            tr = psR.tile([128, 128], BF16, tag="tr")
            nc.tensor.transpose(tr[:], s_sb[:, g * 128:(g + 1) * 128],
                                identb[:])
            sT = sbr.tile([128, 128], BF16, tag="sT")
            nc.scalar.activation(sT[:], tr[:], AF.Copy)
            sTs.append(sT)

        LAG = 4
        for g in range(NG):
            if g % 4 == 0:
                emit_smult(g // 4)
            emit_transpose(g)
            if g >= LAG:
                nc.tensor.matmul(h1_ps[:], sTs[g - LAG][:],
                                 up_sb[:, (g - LAG) * D_MODEL:
                                       (g - LAG + 1) * D_MODEL],
                                 start=(g == LAG), stop=False)
        for g in range(NG - LAG, NG):
            nc.tensor.matmul(h1_ps[:], sTs[g][:],
                             up_sb[:, g * D_MODEL:(g + 1) * D_MODEL],
                             start=False, stop=(g == NG - 1))

        # ---------- combine + layernorm ----------
        A_sb = sb.tile([BL, D_MODEL], F32, tag="A")
        nc.vector.tensor_scalar(A_sb[:], h1_ps[:], rden[:], None,
                                op0=ALU.mult)
    nc.vector.tensor_add(A_sb[:], A_sb[:], h2_sb[:])
    x_sb = sb.tile([BL, D_MODEL], F32, tag="x")
    nc.vector.scalar_tensor_tensor(
        out=x_sb[:], in0=A_sb[:], scalar=float(gamma_f), in1=zb_sb[:],
        op0=ALU.mult, op1=ALU.add)
    mean = sb.tile([BL, 1], F32, tag="mean")
    nc.vector.reduce_sum(mean[:], x_sb[:], axis=mybir.AxisListType.X)
    nc.vector.tensor_scalar_mul(mean[:], mean[:], 1.0 / D_MODEL)
    xc = sb.tile([BL, D_MODEL], F32, tag="xc")
    nc.vector.tensor_scalar(xc[:], x_sb[:], mean[:], None,
                            op0=ALU.subtract)
    xsq = A_sb  # A dead once x is computed
    ssq = sb.tile([BL, 1], F32, tag="ssq")
    nc.scalar.activation(xsq[:], xc[:], AF.Square, accum_out=ssq[:])
    vare = sb.tile([BL, 1], F32, tag="vare")
    nc.vector.tensor_scalar(vare[:], ssq[:], 1.0 / D_MODEL, LN_EPS,
                            op0=ALU.mult, op1=ALU.add)
    sd = sb.tile([BL, 1], F32, tag="sd")
    nc.scalar.activation(sd[:], vare[:], AF.Sqrt)
    rstd = sb.tile([BL, 1], F32, tag="rstd")
    nc.vector.reciprocal(rstd[:], sd[:])
    out_sb = x_sb  # x dead
    nc.vector.scalar_tensor_tensor(
        out=out_sb[:], in0=xc[:], scalar=rstd[:], in1=ls_sb[:],
        op0=ALU.mult, op1=ALU.mult)
    nc.vector.tensor_add(out_sb[:], out_sb[:], lb_sb[:])
    nc.scalar.dma_start(out_d[:], out_sb[:])


def _pack(x, p=128):
    """[K*p, F] row-chunked -> [p, K*F] (chunk k at cols k*F:(k+1)*F)."""
    k = x.shape[0] // p
    return np.ascontiguousarray(
        x.reshape(k, p, -1).transpose(1, 0, 2).reshape(p, -1))


def kernel(z, pool_vectors, W_Q, W_K, aspect_logits, tau,
           W_base, b_base, gamma, ln_scale, ln_bias):
    global LAST_EXEC_NS
    z = np.asarray(z, np.float32)
    pool = np.asarray(pool_vectors, np.float32)
    W_Q = np.asarray(W_Q, np.float32)
    W_K = np.asarray(W_K, np.float32)
    aspect_logits = np.asarray(aspect_logits, np.float32)
    tau_f = float(np.asarray(tau))
    W_base = np.asarray(W_base, np.float32)
    b_base = np.asarray(b_base, np.float32)
    gamma_f = float(np.asarray(gamma))
    ln_scale = np.asarray(ln_scale, np.float32)
    ln_bias = np.asarray(ln_bias, np.float32)

    e = np.exp(aspect_logits - aspect_logits.max())
    w = e / e.sum()
    w0_f, w1_f = float(w[0]), float(w[1])

    nc = _build(tau_f, w0_f, w1_f, gamma_f)

    fp8 = ml_dtypes.float8_e4m3
    bf16 = ml_dtypes.bfloat16

    # ---- shared host-side layout prep ----
    wk_cat = np.concatenate([W_K[0], W_K[1]], axis=1)          # [D, 128]
    wk = _pack((wk_cat * 64.0).astype(fp8))                    # [128, 128*128]
    wq = _pack(np.concatenate([W_Q[0], W_Q[1]], axis=1).astype(bf16))
    # V'^T: [a, n*R + r], scaled
    vt = _pack((pool[:, U_END:V_END].reshape(N, R, D_MODEL)
                .transpose(2, 0, 1).reshape(D_MODEL, NR)
                * SC_V).astype(fp8))
    # up' = [SC_U*U_perm; SC_U*bias] [9216, 512], scaled
    up_rows = np.concatenate([
        pool[:, :U_END].reshape(N, D_MODEL, R).transpose(0, 2, 1)
        .reshape(NR, D_MODEL),
        pool[:, V_END:B_END],
    ], axis=0) * SC_U
    up = _pack(up_rows.astype(fp8))                            # [128, 72*512]
    wbt = _pack(np.ascontiguousarray(W_base.T).astype(bf16))   # [128, 4*512]
    ls = np.broadcast_to(ln_scale, (BL, D_MODEL)).astype(np.float32).copy()
    lb = np.broadcast_to(ln_bias, (BL, D_MODEL)).astype(np.float32).copy()
    gb = (gamma_f * b_base).astype(np.float32)

    in_maps = []
    for c in range(NCORES):
        z_loc = np.ascontiguousarray(z[c * BL:(c + 1) * BL])
        zt_loc = _pack(np.ascontiguousarray(z_loc.T).astype(bf16))
        pk_loc = _pack((np.ascontiguousarray(
            pool[c * NL:(c + 1) * NL, :].T) * 16.0).astype(fp8))
        in_maps.append({
            "pk": pk_loc, "wk": wk, "wq": wq, "zt": zt_loc,
            "zb": z_loc + gb, "ls": ls, "lb": lb, "wbt": wbt,
            "vt": vt, "up": up,
        })

    res = run_bass_kernel_spmd(nc, in_maps, core_ids=list(range(NCORES)),
                               trace=TRACE, tmpdir=TMPDIR)
    LAST_EXEC_NS = res.exec_time_ns
    out = np.concatenate([res.results[c]["out"] for c in range(NCORES)],
                         axis=0)
    return out.astype(np.float32)
